# revision 2
# baseline (speedup 1.0000x reference)
"""NetVLAD Trainium2 kernel — data-parallel over N across 8 cores.

Per core: 4 images [C=128, P=4096].  Pipeline per 1024-pixel chunk:
  PE:   logits[p,k] = x_tile.T @ conv_wT   (x_tile stationary, shared with
        the x-transpose matmul x_tile.T @ I -> xT[p,c]); ssq[p] via
        xsq_tile.T @ ones.
  DVE/POOL/ACT: softmax over k in [pixel-partition, k-free] layout with
        per-pixel scalars held as [128, 8] stat columns and broadcast via
        step-0 access patterns.
  PE:   vlad^[k,c] += a_r.T-style accumulation: lhsT=a_r[:, :56],
        rhs=xT -> psum[56,128]; cluster mass s_k via rhs=n-col.
Final per image: vlad = term1 - s*cen, intra-normalize over k (via PE
transpose), global normalize, write [56,128] rows.
"""

import os
import sys

for _p in ("/opt/trn_rl_repo",):
    if _p not in sys.path:
        sys.path.insert(0, _p)

import numpy as np

NIMG = 4      # images per core
C = 128
K = 64
KE = 56
P = 4096
TPC = 8       # pixel tiles (128 px) per chunk
CH = TPC * 128
NCH = P // CH  # 4 chunks per image

_cache = {}


def _build():
    import concourse.bass as bass
    import concourse.mybir as mybir
    from concourse import bacc, tile

    f32 = mybir.dt.float32
    Alu = mybir.AluOpType
    Act = mybir.ActivationFunctionType

    nc = bacc.Bacc()
    x_in = nc.declare_dram_parameter("x", [NIMG, C, P], f32, isOutput=False)
    # packed consts: wT[0:64] | b8[64:576] | ident[576:704] | ones[704:832]
    # | cen[832:960] (partitions 0:56)
    cst_in = nc.declare_dram_parameter("consts", [C, 960], f32, isOutput=False)
    out_ext = nc.declare_dram_parameter("out", [NIMG, KE, C], f32, isOutput=True)
    dbg_ext = nc.declare_dram_parameter("dbg", [C, 680], f32, isOutput=True)

    with tile.TileContext(nc) as tc:
        with (
            tc.tile_pool(name="const", bufs=1) as cpool,
            tc.tile_pool(name="xin", bufs=3) as xpool,
            tc.tile_pool(name="work", bufs=2) as wpool,
            tc.tile_pool(name="stats", bufs=2) as spool,
            tc.tile_pool(name="fin", bufs=2) as fpool,
            tc.tile_pool(name="psL", bufs=2, space="PSUM") as pL,
            tc.tile_pool(name="psT", bufs=1, space="PSUM") as pT,
            tc.tile_pool(name="psS", bufs=2, space="PSUM") as pS,
            tc.tile_pool(name="psV", bufs=1, space="PSUM") as pV,
            tc.tile_pool(name="psF", bufs=1, space="PSUM") as pF,
        ):
            cst = cpool.tile([C, 960], f32, tag="cst")
            nc.gpsimd.dma_start(cst[:], cst_in[:])
            wT = cst[:, 0:K]
            b8 = cst[:, 64:64 + TPC * K]
            ident = cst[:, 576:576 + C]
            onesc = cst[:, 704:705]
            onesr = cst[0:1, 704:704 + C]
            cen = cst[0:KE, 832:832 + C]

            # PE warm-up: make PE observe the const-DMA semaphore once, so
            # later matmuls need at most one additional wait each.
            warm = pL.tile([C, TPC * K], f32, tag="L")
            nc.tensor.matmul(warm[0:1, 0:1], onesc, onesc,
                             start=True, stop=True)

            for img in range(NIMG):
                # [0:56, 0:128] vlad accum; [0:56, 128:129] s_k accum (via
                # the appended n-column in the rhs). Sole writer of its bank:
                # any other start=True matmul into this bank would clear it.
                psV = pV.tile([C, 160], f32, tag="psV")
                for ch in range(NCH):
                    xin = xpool.tile([C, CH], f32, tag="x")
                    nc.gpsimd.dma_start(xin[:], x_in[img, :, ch * CH:(ch + 1) * CH])
                    xsq = wpool.tile([C, CH], f32, tag="xsq")
                    nc.vector.tensor_mul(xsq[:], xin[:], xin[:])

                    psumL = pL.tile([C, TPC * K], f32, tag="L")
                    psumT = pT.tile([C, CH], f32, tag="T")
                    psumS = pS.tile([C, TPC], f32, tag="S")
                    for j in range(TPC):
                        xt = xin[:, j * 128:(j + 1) * 128]
                        nc.tensor.matmul(psumL[:, j * K:(j + 1) * K], xt, wT,
                                         start=True, stop=True)
                        nc.tensor.matmul(psumT[:, j * 128:(j + 1) * 128], xt,
                                         ident, start=True, stop=True)
                        nc.tensor.matmul(psumS[:, j:j + 1],
                                         xsq[:, j * 128:(j + 1) * 128], onesc,
                                         start=True, stop=True)

                    ncol = spool.tile([C, TPC], f32, tag="ncol")
                    nc.scalar.activation(ncol[:], psumS[:], Act.Sqrt)
                    invc = spool.tile([C, TPC], f32, tag="invc")
                    nc.vector.reciprocal(invc[:], ncol[:])

                    l3 = lambda t: t[:].rearrange("p (t k) -> p t k", k=K)
                    # u = raw * inv_n  (per-pixel scale, bcast along k)
                    lu = wpool.tile([C, TPC * K], f32, tag="lu")
                    nc.vector.tensor_tensor(
                        l3(lu), l3(psumL),
                        invc[:].broadcast_to([C, TPC, K]), Alu.mult)
                    # l = u + b   (bias per-k, pre-tiled 8x from host)
                    ll = wpool.tile([C, TPC * K], f32, tag="ll")
                    nc.vector.tensor_tensor(ll[:], lu[:], b8, Alu.add)
                    # m = max_k l
                    mcol = spool.tile([C, TPC], f32, tag="mcol")
                    nc.vector.tensor_reduce(mcol[:], l3(ll),
                                            axis=mybir.AxisListType.X,
                                            op=Alu.max)
                    # d = l - m
                    dd = wpool.tile([C, TPC * K], f32, tag="dd")
                    nc.vector.tensor_tensor(
                        l3(dd), l3(ll),
                        mcol[:].broadcast_to([C, TPC, K]), Alu.subtract)
                    # e = exp(d)
                    ee = wpool.tile([C, TPC * K], f32, tag="ee")
                    nc.scalar.activation(ee[:], dd[:], Act.Exp)
                    # sumexp
                    scol = spool.tile([C, TPC], f32, tag="scol")
                    nc.vector.tensor_reduce(scol[:], l3(ee),
                                            axis=mybir.AxisListType.X,
                                            op=Alu.add)
                    gcol = spool.tile([C, TPC], f32, tag="gcol")
                    nc.vector.reciprocal(gcol[:], scol[:])
                    rcol = spool.tile([C, TPC], f32, tag="rcol")
                    nc.vector.tensor_tensor(rcol[:], invc[:], gcol[:], Alu.mult)
                    # a_r = e * (inv_n / sumexp)
                    aa = wpool.tile([C, TPC * K], f32, tag="aa")
                    nc.vector.tensor_tensor(
                        l3(aa), l3(ee),
                        rcol[:].broadcast_to([C, TPC, K]), Alu.mult)
                    # xT evict into [x-tile | n-col] interleaved layout so
                    # each vlad rhs is one contiguous [128, 129] slab
                    xTs = wpool.tile([C, TPC * 129], f32, tag="xTs")
                    xTs_v = xTs[:].rearrange("p (t q) -> p t q", q=129)
                    nc.scalar.activation(
                        xTs_v[:, :, 0:128],
                        psumT[:].rearrange("p (t q) -> p t q", q=128),
                        Act.Copy)
                    nc.vector.tensor_copy(
                        xTs_v[:, :, 128:129],
                        ncol[:].broadcast_to([C, TPC, 1]))

                    if img == 0 and ch == 0:
                        nc.gpsimd.dma_start(dbg_ext[:, 0:TPC * K], aa[:])
                        nc.gpsimd.dma_start(dbg_ext[:, 512:512 + TPC], ncol[:])
                        nc.gpsimd.dma_start(dbg_ext[:, 520:520 + TPC], invc[:])
                        nc.gpsimd.dma_start(dbg_ext[:, 528:528 + TPC], mcol[:])
                        nc.gpsimd.dma_start(dbg_ext[:, 536:536 + TPC], scol[:])

                    # PE observer of the ACT semaphore (xTs write), so each
                    # vlad matmul below carries at most one (DVE) wait.
                    nc.tensor.matmul(psumT[0:1, 0:1], xTs[:, 0:1], onesc,
                                     start=True, stop=True)

                    first = ch == 0
                    last = ch == NCH - 1
                    for j in range(TPC):
                        nc.tensor.matmul(psV[0:KE, 0:129],
                                         aa[:, j * K:j * K + KE],
                                         xTs[:, j * 129:(j + 1) * 129],
                                         start=(first and j == 0),
                                         stop=(last and j == TPC - 1))

                # ---- per-image tail ----
                ps = pF.tile([C, 192], f32, tag="psF")
                negs = spool.tile([KE, 1], f32, tag="negs")
                nc.vector.tensor_scalar_mul(negs[:], psV[0:KE, 128:129], -1.0)
                vk = fpool.tile([KE, C], f32, tag="vk")
                nc.vector.scalar_tensor_tensor(vk[:], cen, negs[:],
                                               psV[0:KE, 0:C],
                                               Alu.mult, Alu.add)
                if img == 0:
                    nc.gpsimd.dma_start(
                        dbg_ext[0:KE, 544:544 + C], vk[:])
                    nc.gpsimd.dma_start(
                        dbg_ext[0:KE, 672:673], negs[:])
                # transpose -> [c, k]
                nc.tensor.matmul(ps[:, 0:KE], vk[:], ident[0:KE, 0:KE],
                                 start=True, stop=True)
                trash = fpool.tile([C, KE], f32, tag="trash")
                ssqk = spool.tile([C, 1], f32, tag="ssqk")
                nc.scalar.activation(trash[:], ps[:, 0:KE], Act.Square,
                                     accum_out=ssqk[:])
                nk = spool.tile([C, 1], f32, tag="nk")
                nc.scalar.activation(nk[:], ssqk[:], Act.Sqrt)
                nkc = spool.tile([C, 1], f32, tag="nkc")
                nc.vector.tensor_scalar_max(nkc[:], nk[:], 1e-12)
                invk = spool.tile([C, 1], f32, tag="invk")
                nc.vector.reciprocal(invk[:], nkc[:])
                t2 = spool.tile([C, 1], f32, tag="t2")
                nc.vector.scalar_tensor_tensor(t2[:], ssqk[:], invk[:], invk[:],
                                               Alu.mult, Alu.mult)
                # scalar matmuls go to a separate bank (start=True clears the
                # whole target bank, and ps[:, 0:KE] is still live)
                tiny = pL.tile([C, TPC * K], f32, tag="L")
                nc.tensor.matmul(tiny[0:1, 0:1], t2[:], onesc,
                                 start=True, stop=True)
                tot = spool.tile([1, 1], f32, tag="tot")
                nc.scalar.activation(tot[:], tiny[0:1, 0:1], Act.Sqrt)
                totc = spool.tile([1, 1], f32, tag="totc")
                nc.vector.tensor_scalar_max(totc[:], tot[:], 1e-12)
                fv = spool.tile([1, 1], f32, tag="fv")
                nc.vector.reciprocal(fv[:], totc[:])
                # broadcast fv to [128,1] via PE (wipes the tiny bank again;
                # tot was already evicted to SBUF)
                nc.tensor.matmul(tiny[:, 2:3], onesr, fv[:],
                                 start=True, stop=True)
                comb = spool.tile([C, 1], f32, tag="comb")
                nc.vector.tensor_tensor(comb[:], invk[:], tiny[:, 2:3], Alu.mult)
                vnT = fpool.tile([C, KE], f32, tag="vnT")
                nc.vector.tensor_scalar(vnT[:], ps[:, 0:KE], comb[:], None,
                                        Alu.mult)
                # transpose back -> [k, c]
                nc.tensor.matmul(ps[0:KE, 64:64 + C], vnT[:], ident,
                                 start=True, stop=True)
                ob = fpool.tile([KE, C], f32, tag="ob")
                nc.scalar.activation(ob[:], ps[0:KE, 64:64 + C], Act.Copy)
                nc.gpsimd.dma_start(out_ext[img], ob[:])

    nc.compile()
    return nc


def _get_nc():
    if "nc" not in _cache:
        _cache["nc"] = _build()
    return _cache["nc"]


def _make_in_maps(x, conv_w, conv_b, centroids, n_cores=8):
    x = np.asarray(x, dtype=np.float32)
    conv_w = np.asarray(conv_w, dtype=np.float32)
    conv_b = np.asarray(conv_b, dtype=np.float32)
    centroids = np.asarray(centroids, dtype=np.float32)

    N = x.shape[0]
    per = N // n_cores
    assert per == NIMG

    xr = x.reshape(N, C, P)
    cst = np.zeros((C, 960), dtype=np.float32)
    cst[:, 0:K] = conv_w.T
    cst[:, 64:64 + TPC * K] = np.tile(conv_b, TPC)[None, :]
    cst[:, 576:576 + C] = np.eye(C, dtype=np.float32)
    cst[:, 704:832] = 1.0
    cst[0:KE, 832:832 + C] = centroids[:KE]

    in_maps = []
    for i in range(n_cores):
        in_maps.append({
            "x": np.ascontiguousarray(xr[i * per:(i + 1) * per]),
            "consts": cst,
        })
    return in_maps


def kernel(x, conv_w, conv_b, centroids):
    from concourse.bass_utils import run_bass_kernel_spmd

    n_cores = 8
    per = np.asarray(x).shape[0] // n_cores
    in_maps = _make_in_maps(x, conv_w, conv_b, centroids, n_cores)

    nc = _get_nc()
    res = run_bass_kernel_spmd(nc, in_maps, list(range(n_cores)))
    outs = [np.asarray(r["out"]).reshape(per, KE * C) for r in res.results]
    return np.concatenate(outs, axis=0)


if __name__ == "__main__":
    rng = np.random.default_rng(0)
    x = rng.standard_normal((32, C, 64, 64), dtype=np.float32)
    w = rng.standard_normal((K, C), dtype=np.float32)
    b = rng.standard_normal((K,), dtype=np.float32)
    c = rng.random((K, C), dtype=np.float32)
    out = kernel(x=x, conv_w=w, conv_b=b, centroids=c)
    print(out.shape, out.dtype)



# revision 15
# speedup vs baseline: 1.0675x; 1.0675x over previous
"""NetVLAD Trainium2 kernel — data-parallel over N across 8 cores, bf16.

Per core: 4 images [C=128, P=4096].  x is loaded twice per 1024-px chunk:
once plain [c, p] (logits stationaries) and once via HWDGE DMA-transpose
[p, c] straight into SBUF (no PE transpose, no psum evict).

Per chunk (8 tiles of 128 px):
  SP:   plain DMA + transpose DMA.
  Pool: xsq = xT*xT; rcol = invn/sumexp; a_r = ee * rcol.
  DVE:  ssq 3D-reduce; fused nll=-(u+b) with accum min -> -max(l); recip.
  ACT:  invn/n via exp(+-0.5*ln(ssq)) (ln+exp share one act table -> no
        table thrash; n lands directly in the xTs 129th column);
        u = raw*invn scaled-copy from psum; ee = exp(-nll - m) + sumexp.
  PE:   8x logits matmul (stationary x_tile bf16, moving wT 64 cols),
        8x vlad matmul (stationary a_r[:, :56], moving [xT | n] 129 cols)
        accumulated into psum [56, 129] over the image.
Tail per image: vk = term1 - s*cen, PE transpose, intra-norm via
exp(-0.5 ln(ssq_k)) * 1/sqrt(128) (the global norm is exactly 1/sqrt(128)
because every intra-normalized column has unit norm), transpose back, DMA.
"""

import sys

for _p in ("/opt/trn_rl_repo",):
    if _p not in sys.path:
        sys.path.insert(0, _p)

import numpy as np
import ml_dtypes

NIMG = 4      # images per core
C = 128
K = 64
KE = 56
P = 4096
TPC = 8       # 128-px tiles per chunk
CH = TPC * 128
NCH = P // CH  # 4 chunks per image
TS = 144      # xTs per-tile stride (bf16 elems); 288B = 32B-aligned

_cache = {}


def _build():
    import concourse.mybir as mybir
    from concourse import bacc, tile

    f32 = mybir.dt.float32
    bf16 = mybir.dt.bfloat16
    Alu = mybir.AluOpType
    Act = mybir.ActivationFunctionType

    nc = bacc.Bacc()
    x_in = nc.declare_dram_parameter("x", [NIMG, C, P], bf16, isOutput=False)
    cb_in = nc.declare_dram_parameter("cstb", [C, K], bf16, isOutput=False)
    # f32 consts: b64 [0:64] | identF [64:192] | cen rows0:56 [192:320]
    # | ln(1/sqrt(128)) [320]
    cf_in = nc.declare_dram_parameter("cstf", [C, 840], f32, isOutput=False)
    out_ext = nc.declare_dram_parameter("out", [NIMG, KE, C], f32, isOutput=True)
    dbg_ext = nc.declare_dram_parameter("dbg", [C, 704], f32, isOutput=True)

    with tile.TileContext(nc) as tc:
        with (
            tc.tile_pool(name="const", bufs=1) as cpool,
            tc.tile_pool(name="xin", bufs=3) as xpool,
            tc.tile_pool(name="xts", bufs=3) as tpool,
            tc.tile_pool(name="work", bufs=2) as wpool,
            tc.tile_pool(name="stats", bufs=2) as spool,
            tc.tile_pool(name="fin", bufs=2) as fpool,
            tc.tile_pool(name="psL", bufs=3, space="PSUM") as pL,
            tc.tile_pool(name="psF", bufs=1, space="PSUM") as pF,
            tc.tile_pool(name="psV", bufs=2, space="PSUM") as pV,
        ):
            wT = cpool.tile([C, K], bf16, tag="wT")
            cstf = cpool.tile([C, 840], f32, tag="cstf")
            nc.sync.dma_start(wT[:], cb_in[:])
            nc.sync.dma_start(cstf[:], cf_in[:])
            b64 = cstf[:, 0:K]
            identF = cstf[:, 64:192]
            cen = cstf[0:KE, 192:320]
            gnl = cstf[:, 320:321]
            b512 = cstf[:, 328:328 + TPC * K]

            for img in range(NIMG):
                # vlad accumulator [0:56, 0:128]=term1, [:,128]=s_k
                psV = pV.tile([C, 512], f32, tag="psV")
                for ch in range(NCH):
                    xin = xpool.tile([C, CH], bf16, tag="x")
                    nc.sync.dma_start(xin[:], x_in[img, :, ch * CH:(ch + 1) * CH])

                    # xT via HWDGE dma transpose: [p, (t, TS): 0:128]
                    xTs = tpool.tile([C, TPC * TS], bf16, tag="xTs")
                    xTs_v = xTs[:].rearrange("p (t r) -> p t r", r=TS)
                    nc.sync.dma_start_transpose(
                        xTs_v[:, :, 0:128],
                        x_in[img, :, ch * CH:(ch + 1) * CH])

                    # ssq: square on Pool, 3D-reduce on DVE
                    xsq = wpool.tile([C, TPC * 128], bf16, tag="xsq")
                    xsq_v = xsq[:].rearrange("p (t r) -> p t r", r=128)
                    nc.vector.tensor_tensor(
                        xsq_v, xTs_v[:, :, 0:128], xTs_v[:, :, 0:128], Alu.mult)
                    ssq = spool.tile([C, TPC], f32, tag="ssq")
                    nc.vector.tensor_reduce(
                        ssq[:], xsq_v, axis=mybir.AxisListType.X, op=Alu.add)

                    # invn = exp(-0.5 ln ssq); n straight into xTs col 128
                    lnssq = spool.tile([C, TPC], f32, tag="lnssq")
                    nc.scalar.activation(lnssq[:], ssq[:], Act.Ln)
                    invc = spool.tile([C, TPC], f32, tag="invc")
                    nc.scalar.activation(invc[:], lnssq[:], Act.Exp, scale=-0.5)
                    nc.scalar.activation(xTs_v[:, :, 128:129], lnssq[:],
                                         Act.Exp, scale=0.5)

                    # logits: stationary x_tile, moving wT (64 cols)
                    psumL = pL.tile([C, TPC * K], f32, tag="L")
                    for j in range(TPC):
                        nc.tensor.matmul(psumL[:, j * K:(j + 1) * K],
                                         xin[:, j * 128:(j + 1) * 128], wT[:],
                                         start=True, stop=True)

                    # u = raw*invn (ACT scaled copy, psum -> SBUF f32)
                    uu = wpool.tile([C, TPC * K], f32, tag="uu")
                    for j in range(TPC):
                        nc.scalar.activation(uu[:, j * K:(j + 1) * K],
                                             psumL[:, j * K:(j + 1) * K],
                                             Act.Copy, scale=invc[:, j:j + 1])

                    # ll = u + b (DVE, b broadcast over tiles)
                    ll = wpool.tile([C, TPC * K], f32, tag="ll")
                    nc.vector.tensor_tensor(ll[:], uu[:], b512, Alu.add)

                    # -max over k per (pixel, tile)
                    mcol = spool.tile([C, TPC], f32, tag="mcol")
                    nc.vector.tensor_reduce(
                        mcol[:], ll[:].rearrange("p (t k) -> p t k", k=K),
                        axis=mybir.AxisListType.X, op=Alu.max)
                    nmcol = spool.tile([C, TPC], f32, tag="nmcol")
                    nc.vector.tensor_scalar_mul(nmcol[:], mcol[:], -1.0)

                    # ee = exp(l - m), scol = sumexp  (ACT per tile)
                    ee = wpool.tile([C, TPC * K], bf16, tag="ee")
                    scol = spool.tile([C, TPC], f32, tag="scol")
                    for j in range(TPC):
                        nc.scalar.activation(
                            ee[:, j * K:(j + 1) * K], ll[:, j * K:(j + 1) * K],
                            Act.Exp, bias=nmcol[:, j:j + 1],
                            accum_out=scol[:, j:j + 1])

                    gcol = spool.tile([C, TPC], f32, tag="gcol")
                    nc.vector.reciprocal(gcol[:], scol[:])
                    rcol = spool.tile([C, TPC], f32, tag="rcol")
                    nc.vector.tensor_tensor(rcol[:], invc[:], gcol[:], Alu.mult)

                    # a_r = ee * rcol (Pool, broadcast over k)
                    aa = wpool.tile([C, TPC * K], bf16, tag="aa")
                    nc.vector.tensor_tensor(
                        aa[:].rearrange("p (t k) -> p t k", k=K),
                        ee[:].rearrange("p (t k) -> p t k", k=K),
                        rcol[:].broadcast_to([C, TPC, K]), Alu.mult)

                    if img == 0 and ch == 0:
                        nc.gpsimd.dma_start(dbg_ext[:, 0:TPC * K], aa[:])
                        nc.sync.dma_start(dbg_ext[:, 512:512 + TPC], ssq[:])
                        nc.sync.dma_start(dbg_ext[:, 520:520 + TPC], invc[:])
                        nc.sync.dma_start(dbg_ext[:, 528:528 + TPC], nmcol[:])
                        nc.sync.dma_start(dbg_ext[:, 536:536 + TPC], scol[:])
                        nc.gpsimd.dma_start(dbg_ext[:, 544:544 + 129],
                                            xTs_v[0:C, 0, 0:129])

                    first = ch == 0
                    last = ch == NCH - 1
                    for j in range(TPC):
                        nc.tensor.matmul(psV[0:KE, 0:129],
                                         aa[:, j * K:j * K + KE],
                                         xTs[:, j * TS:j * TS + 129],
                                         start=(first and j == 0),
                                         stop=(last and j == TPC - 1))

                # ---- per-image tail ----
                negs = spool.tile([KE, 1], f32, tag="negs")
                nc.vector.tensor_scalar_mul(negs[:], psV[0:KE, 128:129], -1.0)
                vk = fpool.tile([KE, C], f32, tag="vk")
                nc.vector.scalar_tensor_tensor(vk[:], cen, negs[:],
                                               psV[0:KE, 0:C],
                                               Alu.mult, Alu.add)
                if img == 0:
                    nc.sync.dma_start(dbg_ext[0:KE, 680:681], negs[:])
                # transpose -> [c, k]
                psA = pF.tile([C, 256], f32, tag="psA")
                nc.tensor.matmul(psA[:, 0:KE], vk[:], identF[0:KE, 0:KE],
                                 is_transpose=True, start=True, stop=True)
                trash_a = fpool.tile([C, KE], f32, tag="tra")
                ssqk = spool.tile([C, 1], f32, tag="ssqk")
                nc.scalar.activation(trash_a[:], psA[:, 0:KE], Act.Square,
                                     accum_out=ssqk[:])
                lnk = spool.tile([C, 1], f32, tag="lnk")
                nc.scalar.activation(lnk[:], ssqk[:], Act.Ln)
                # comb = 1/sqrt(ssq_k) * 1/sqrt(128)
                comb = spool.tile([C, 1], f32, tag="comb")
                nc.scalar.activation(comb[:], lnk[:], Act.Exp,
                                     scale=-0.5, bias=gnl)
                vnT = fpool.tile([C, KE], f32, tag="vnT")
                nc.vector.tensor_scalar(vnT[:], psA[:, 0:KE], comb[:], None,
                                        Alu.mult)
                # transpose back -> [k, c]
                psB = pF.tile([C, 256], f32, tag="psB")
                nc.tensor.matmul(psB[0:KE, 0:C], vnT[:], identF,
                                 is_transpose=True, start=True, stop=True)
                ob = fpool.tile([KE, C], f32, tag="ob")
                nc.scalar.activation(ob[:], psB[0:KE, 0:C], Act.Copy)
                nc.sync.dma_start(out_ext[img], ob[:])

    nc.compile()
    return nc


def _get_nc():
    if "nc" not in _cache:
        _cache["nc"] = _build()
    return _cache["nc"]


def _make_in_maps(x, conv_w, conv_b, centroids, n_cores=8):
    x = np.asarray(x, dtype=np.float32)
    conv_w = np.asarray(conv_w, dtype=np.float32)
    conv_b = np.asarray(conv_b, dtype=np.float32)
    centroids = np.asarray(centroids, dtype=np.float32)

    N = x.shape[0]
    per = N // n_cores
    assert per == NIMG

    xr = np.ascontiguousarray(
        x.reshape(N, C, P).astype(ml_dtypes.bfloat16))

    cstb = np.ascontiguousarray(conv_w.T.astype(ml_dtypes.bfloat16))

    cstf = np.zeros((C, 840), dtype=np.float32)
    cstf[:, 0:K] = conv_b[None, :]
    cstf[:, 64:192] = np.eye(C, dtype=np.float32)
    cstf[0:KE, 192:320] = centroids[:KE]
    cstf[:, 320] = -0.5 * np.log(128.0)
    cstf[:, 328:328 + 512] = np.tile(conv_b, 8)[None, :]

    in_maps = []
    for i in range(n_cores):
        in_maps.append({
            "x": np.ascontiguousarray(xr[i * per:(i + 1) * per]),
            "cstb": cstb,
            "cstf": cstf,
        })
    return in_maps


def kernel(x, conv_w, conv_b, centroids):
    from concourse.bass_utils import run_bass_kernel_spmd

    n_cores = 8
    per = np.asarray(x).shape[0] // n_cores
    in_maps = _make_in_maps(x, conv_w, conv_b, centroids, n_cores)

    nc = _get_nc()
    res = run_bass_kernel_spmd(nc, in_maps, list(range(n_cores)))
    outs = [np.asarray(r["out"]).reshape(per, KE * C) for r in res.results]
    return np.concatenate(outs, axis=0)


if __name__ == "__main__":
    rng = np.random.default_rng(0)
    x = rng.standard_normal((32, C, 64, 64), dtype=np.float32)
    w = rng.standard_normal((K, C), dtype=np.float32)
    b = rng.standard_normal((K,), dtype=np.float32)
    c = rng.random((K, C), dtype=np.float32)
    out = kernel(x=x, conv_w=w, conv_b=b, centroids=c)
    print(out.shape, out.dtype)


# revision 16
# speedup vs baseline: 1.5347x; 1.4377x over previous
"""NetVLAD Trainium2 kernel — data-parallel over N across 8 cores, bf16.

Per core: 4 images [C=128, P=4096].  x is loaded twice per 1024-px chunk:
once plain [c, p] (logits stationaries) and once via HWDGE DMA-transpose
[p, c] straight into SBUF (no PE transpose, no psum evict).

Per chunk (8 tiles of 128 px):
  SP:   plain DMA + transpose DMA.
  Pool: xsq = xT*xT; rcol = invn/sumexp; a_r = ee * rcol.
  DVE:  ssq 3D-reduce; fused nll=-(u+b) with accum min -> -max(l); recip.
  ACT:  invn/n via exp(+-0.5*ln(ssq)) (ln+exp share one act table -> no
        table thrash; n lands directly in the xTs 129th column);
        u = raw*invn scaled-copy from psum; ee = exp(-nll - m) + sumexp.
  PE:   8x logits matmul (stationary x_tile bf16, moving wT 64 cols),
        8x vlad matmul (stationary a_r[:, :56], moving [xT | n] 129 cols)
        accumulated into psum [56, 129] over the image.
Tail per image: vk = term1 - s*cen, PE transpose, intra-norm via
exp(-0.5 ln(ssq_k)) * 1/sqrt(128) (the global norm is exactly 1/sqrt(128)
because every intra-normalized column has unit norm), transpose back, DMA.
"""

import sys

for _p in ("/opt/trn_rl_repo",):
    if _p not in sys.path:
        sys.path.insert(0, _p)

import numpy as np
import ml_dtypes

NIMG = 4      # images per core
C = 128
K = 64
KE = 56
P = 4096
TPC = 8       # 128-px tiles per chunk
CH = TPC * 128
NCH = P // CH  # 4 chunks per image
TS = 144      # xTs per-tile stride (bf16 elems); 288B = 32B-aligned

_cache = {}


def _build():
    import concourse.mybir as mybir
    from concourse import bacc, tile

    f32 = mybir.dt.float32
    bf16 = mybir.dt.bfloat16
    Alu = mybir.AluOpType
    Act = mybir.ActivationFunctionType

    nc = bacc.Bacc()
    x_in = nc.declare_dram_parameter("x", [NIMG, C, P], bf16, isOutput=False)
    cb_in = nc.declare_dram_parameter("cstb", [C, K], bf16, isOutput=False)
    # f32 consts: b64 [0:64] | identF [64:192] | cen rows0:56 [192:320]
    # | ln(1/sqrt(128)) [320]
    cf_in = nc.declare_dram_parameter("cstf", [C, 840], f32, isOutput=False)
    out_ext = nc.declare_dram_parameter("out", [NIMG, KE, C], f32, isOutput=True)
    dbg_ext = nc.declare_dram_parameter("dbg", [C, 704], f32, isOutput=True)

    with tile.TileContext(nc) as tc:
        with (
            tc.tile_pool(name="const", bufs=1) as cpool,
            tc.tile_pool(name="xin", bufs=3) as xpool,
            tc.tile_pool(name="xts", bufs=2) as tpool,
            tc.tile_pool(name="work", bufs=2) as wpool,
            tc.tile_pool(name="stats", bufs=2) as spool,
            tc.tile_pool(name="fin", bufs=2) as fpool,
            tc.tile_pool(name="psL", bufs=3, space="PSUM") as pL,
            tc.tile_pool(name="psF", bufs=1, space="PSUM") as pF,
            tc.tile_pool(name="psV", bufs=2, space="PSUM") as pV,
        ):
            wT = cpool.tile([C, K], bf16, tag="wT")
            cstf = cpool.tile([C, 840], f32, tag="cstf")
            nc.sync.dma_start(wT[:], cb_in[:])
            nc.sync.dma_start(cstf[:], cf_in[:])
            b64 = cstf[:, 0:K]
            identF = cstf[:, 64:192]
            cen = cstf[0:KE, 192:320]
            gnl = cstf[:, 320:321]
            b512 = cstf[:, 328:328 + TPC * K]

            PT = NCH * TPC  # 32 pixel tiles per image

            for img in range(NIMG):
                # image-level xT store: [p, (32 tiles, TS): 0:128]=xT, col 128=n
                xTs = tpool.tile([C, PT * TS], bf16, tag="xTs")
                xTs_v = xTs[:].rearrange("p (t r) -> p t r", r=TS)
                ssqI = spool.tile([C, PT], f32, tag="ssqI")
                for ch in range(NCH):
                    nc.sync.dma_start_transpose(
                        xTs_v[:, ch * TPC:(ch + 1) * TPC, 0:128],
                        x_in[img, :, ch * CH:(ch + 1) * CH])
                    xsq = wpool.tile([C, CH], bf16, tag="xsq")
                    xsq_v = xsq[:].rearrange("p (t r) -> p t r", r=128)
                    nc.gpsimd.tensor_tensor(
                        xsq_v, xTs_v[:, ch * TPC:(ch + 1) * TPC, 0:128],
                        xTs_v[:, ch * TPC:(ch + 1) * TPC, 0:128], Alu.mult)
                    nc.vector.tensor_reduce(
                        ssqI[:, ch * TPC:(ch + 1) * TPC], xsq_v,
                        axis=mybir.AxisListType.X, op=Alu.add)

                # per-image stats on ACT: 3 ops, one table-set pair
                lnssq = spool.tile([C, PT], f32, tag="lnssq")
                nc.scalar.activation(lnssq[:], ssqI[:], Act.Ln)
                invcI = spool.tile([C, PT], f32, tag="invcI")
                nc.scalar.activation(invcI[:], lnssq[:], Act.Exp, scale=-0.5)
                nc.scalar.activation(xTs_v[:, :, 128:129], lnssq[:],
                                     Act.Exp, scale=0.5)

                # vlad accumulator [0:56, 0:128]=term1, [:,128]=s_k
                psV = pV.tile([C, 512], f32, tag="psV")
                for ch in range(NCH):
                    xin = xpool.tile([C, CH], bf16, tag="x")
                    nc.sync.dma_start(xin[:], x_in[img, :, ch * CH:(ch + 1) * CH])

                    invc = invcI[:, ch * TPC:(ch + 1) * TPC]

                    psumL = pL.tile([C, TPC * K], f32, tag="L")
                    for j in range(TPC):
                        nc.tensor.matmul(psumL[:, j * K:(j + 1) * K],
                                         xin[:, j * 128:(j + 1) * 128], wT[:],
                                         start=True, stop=True)

                    # lu = raw*invn (DVE big op from psum)
                    lu = wpool.tile([C, TPC * K], f32, tag="lu")
                    nc.vector.tensor_tensor(
                        lu[:].rearrange("p (t k) -> p t k", k=K),
                        psumL[:].rearrange("p (t k) -> p t k", k=K),
                        invc.broadcast_to([C, TPC, K]), Alu.mult)
                    # ll = lu + b (Pool)
                    ll = wpool.tile([C, TPC * K], f32, tag="ll")
                    nc.gpsimd.tensor_tensor(ll[:], lu[:], b512, Alu.add)
                    # -max over k per (pixel, tile)
                    nmcol = spool.tile([C, TPC], f32, tag="nmcol")
                    nc.vector.tensor_reduce(
                        nmcol[:], ll[:].rearrange("p (t k) -> p t k", k=K),
                        axis=mybir.AxisListType.X, op=Alu.max, negate=True)
                    # dd = ll - m (Pool)
                    dd = wpool.tile([C, TPC * K], f32, tag="dd")
                    nc.gpsimd.tensor_tensor(
                        dd[:].rearrange("p (t k) -> p t k", k=K),
                        ll[:].rearrange("p (t k) -> p t k", k=K),
                        nmcol[:].broadcast_to([C, TPC, K]), Alu.add)
                    # ee = exp(dd) one big ACT op
                    ee = wpool.tile([C, TPC * K], bf16, tag="ee")
                    nc.scalar.activation(ee[:], dd[:], Act.Exp)
                    # scol = sumexp (DVE)
                    scol = spool.tile([C, TPC], f32, tag="scol")
                    nc.vector.tensor_reduce(
                        scol[:], ee[:].rearrange("p (t k) -> p t k", k=K),
                        axis=mybir.AxisListType.X, op=Alu.add)
                    gcol = spool.tile([C, TPC], f32, tag="gcol")
                    nc.vector.reciprocal(gcol[:], scol[:])
                    rcol = spool.tile([C, TPC], f32, tag="rcol")
                    nc.gpsimd.tensor_tensor(rcol[:], invc, gcol[:], Alu.mult)
                    # a_r = ee * rcol (Pool, broadcast over k)
                    aa = wpool.tile([C, TPC * K], bf16, tag="aa")
                    nc.gpsimd.tensor_tensor(
                        aa[:].rearrange("p (t k) -> p t k", k=K),
                        ee[:].rearrange("p (t k) -> p t k", k=K),
                        rcol[:].broadcast_to([C, TPC, K]), Alu.mult)

                    if img == 0 and ch == 0:
                        nc.gpsimd.dma_start(dbg_ext[:, 0:TPC * K], aa[:])
                        nc.sync.dma_start(dbg_ext[:, 512:512 + TPC],
                                          ssqI[:, 0:TPC])
                        nc.sync.dma_start(dbg_ext[:, 528:528 + TPC], nmcol[:])
                        nc.sync.dma_start(dbg_ext[:, 536:536 + TPC], scol[:])

                    first = ch == 0
                    last = ch == NCH - 1
                    for j in range(TPC):
                        t = ch * TPC + j
                        nc.tensor.matmul(psV[0:KE, 0:129],
                                         aa[:, j * K:j * K + KE],
                                         xTs[:, t * TS:t * TS + 129],
                                         start=(first and j == 0),
                                         stop=(last and j == TPC - 1))

                # ---- per-image tail ----
                negs = spool.tile([KE, 1], f32, tag="negs")
                nc.vector.tensor_scalar_mul(negs[:], psV[0:KE, 128:129], -1.0)
                vk = fpool.tile([KE, C], f32, tag="vk")
                nc.vector.scalar_tensor_tensor(vk[:], cen, negs[:],
                                               psV[0:KE, 0:C],
                                               Alu.mult, Alu.add)
                if img == 0:
                    nc.sync.dma_start(dbg_ext[0:KE, 680:681], negs[:])
                # transpose -> [c, k]
                psA = pF.tile([C, 256], f32, tag="psA")
                nc.tensor.matmul(psA[:, 0:KE], vk[:], identF[0:KE, 0:KE],
                                 is_transpose=True, start=True, stop=True)
                trash_a = fpool.tile([C, KE], f32, tag="tra")
                ssqk = spool.tile([C, 1], f32, tag="ssqk")
                nc.scalar.activation(trash_a[:], psA[:, 0:KE], Act.Square,
                                     accum_out=ssqk[:])
                lnk = spool.tile([C, 1], f32, tag="lnk")
                nc.scalar.activation(lnk[:], ssqk[:], Act.Ln)
                # comb = 1/sqrt(ssq_k) * 1/sqrt(128)
                comb = spool.tile([C, 1], f32, tag="comb")
                nc.scalar.activation(comb[:], lnk[:], Act.Exp,
                                     scale=-0.5, bias=gnl)
                vnT = fpool.tile([C, KE], f32, tag="vnT")
                nc.vector.tensor_scalar(vnT[:], psA[:, 0:KE], comb[:], None,
                                        Alu.mult)
                # transpose back -> [k, c]
                psB = pF.tile([C, 256], f32, tag="psB")
                nc.tensor.matmul(psB[0:KE, 0:C], vnT[:], identF,
                                 is_transpose=True, start=True, stop=True)
                ob = fpool.tile([KE, C], f32, tag="ob")
                nc.scalar.activation(ob[:], psB[0:KE, 0:C], Act.Copy)
                nc.sync.dma_start(out_ext[img], ob[:])

    nc.compile()
    return nc


def _get_nc():
    if "nc" not in _cache:
        _cache["nc"] = _build()
    return _cache["nc"]


def _make_in_maps(x, conv_w, conv_b, centroids, n_cores=8):
    x = np.asarray(x, dtype=np.float32)
    conv_w = np.asarray(conv_w, dtype=np.float32)
    conv_b = np.asarray(conv_b, dtype=np.float32)
    centroids = np.asarray(centroids, dtype=np.float32)

    N = x.shape[0]
    per = N // n_cores
    assert per == NIMG

    xr = np.ascontiguousarray(
        x.reshape(N, C, P).astype(ml_dtypes.bfloat16))

    cstb = np.ascontiguousarray(conv_w.T.astype(ml_dtypes.bfloat16))

    cstf = np.zeros((C, 840), dtype=np.float32)
    cstf[:, 0:K] = conv_b[None, :]
    cstf[:, 64:192] = np.eye(C, dtype=np.float32)
    cstf[0:KE, 192:320] = centroids[:KE]
    cstf[:, 320] = -0.5 * np.log(128.0)
    cstf[:, 328:328 + 512] = np.tile(conv_b, 8)[None, :]

    in_maps = []
    for i in range(n_cores):
        in_maps.append({
            "x": np.ascontiguousarray(xr[i * per:(i + 1) * per]),
            "cstb": cstb,
            "cstf": cstf,
        })
    return in_maps


def kernel(x, conv_w, conv_b, centroids):
    from concourse.bass_utils import run_bass_kernel_spmd

    n_cores = 8
    per = np.asarray(x).shape[0] // n_cores
    in_maps = _make_in_maps(x, conv_w, conv_b, centroids, n_cores)

    nc = _get_nc()
    res = run_bass_kernel_spmd(nc, in_maps, list(range(n_cores)))
    outs = [np.asarray(r["out"]).reshape(per, KE * C) for r in res.results]
    return np.concatenate(outs, axis=0)


if __name__ == "__main__":
    rng = np.random.default_rng(0)
    x = rng.standard_normal((32, C, 64, 64), dtype=np.float32)
    w = rng.standard_normal((K, C), dtype=np.float32)
    b = rng.standard_normal((K,), dtype=np.float32)
    c = rng.random((K, C), dtype=np.float32)
    out = kernel(x=x, conv_w=w, conv_b=b, centroids=c)
    print(out.shape, out.dtype)


# revision 19
# speedup vs baseline: 1.8296x; 1.1922x over previous
"""NetVLAD Trainium2 kernel — data-parallel over N across 8 cores, bf16.

Per core: 4 images [C=128, P=4096].  x is loaded twice per 1024-px chunk:
once plain [c, p] (logits stationaries) and once via HWDGE DMA-transpose
[p, c] straight into SBUF (no PE transpose, no psum evict).

Per chunk (8 tiles of 128 px):
  SP:   plain DMA + transpose DMA.
  Pool: xsq = xT*xT; rcol = invn/sumexp; a_r = ee * rcol.
  DVE:  ssq 3D-reduce; fused nll=-(u+b) with accum min -> -max(l); recip.
  ACT:  invn/n via exp(+-0.5*ln(ssq)) (ln+exp share one act table -> no
        table thrash; n lands directly in the xTs 129th column);
        u = raw*invn scaled-copy from psum; ee = exp(-nll - m) + sumexp.
  PE:   8x logits matmul (stationary x_tile bf16, moving wT 64 cols),
        8x vlad matmul (stationary a_r[:, :56], moving [xT | n] 129 cols)
        accumulated into psum [56, 129] over the image.
Tail per image: vk = term1 - s*cen, PE transpose, intra-norm via
exp(-0.5 ln(ssq_k)) * 1/sqrt(128) (the global norm is exactly 1/sqrt(128)
because every intra-normalized column has unit norm), transpose back, DMA.
"""

import sys

for _p in ("/opt/trn_rl_repo",):
    if _p not in sys.path:
        sys.path.insert(0, _p)

import numpy as np
import ml_dtypes

NIMG = 4      # images per core
C = 128
K = 64
KE = 56
P = 4096
TPC = 8       # 128-px tiles per chunk
CH = TPC * 128
NCH = P // CH  # 4 chunks per image
TS = 144      # xTs per-tile stride (bf16 elems); 288B = 32B-aligned

_cache = {}


def _build():
    import concourse.mybir as mybir
    from concourse import bacc, tile

    f32 = mybir.dt.float32
    bf16 = mybir.dt.bfloat16
    Alu = mybir.AluOpType
    Act = mybir.ActivationFunctionType

    nc = bacc.Bacc()
    x_in = nc.declare_dram_parameter("x", [NIMG, C, P], bf16, isOutput=False)
    cb_in = nc.declare_dram_parameter("cstb", [C, K], bf16, isOutput=False)
    # f32 consts: b64 [0:64] | identF [64:192] | cen rows0:56 [192:320]
    # | ln(1/sqrt(128)) [320]
    cf_in = nc.declare_dram_parameter("cstf", [C, 840], f32, isOutput=False)
    out_ext = nc.declare_dram_parameter("out", [NIMG, KE, C], f32, isOutput=True)
    dbg_ext = nc.declare_dram_parameter("dbg", [C, 704], f32, isOutput=True)

    with tile.TileContext(nc) as tc:
        with (
            tc.tile_pool(name="const", bufs=1) as cpool,
            tc.tile_pool(name="xin", bufs=3) as xpool,
            tc.tile_pool(name="xts", bufs=2) as tpool,
            tc.tile_pool(name="work", bufs=2) as wpool,
            tc.tile_pool(name="stats", bufs=2) as spool,
            tc.tile_pool(name="fin", bufs=2) as fpool,
            tc.tile_pool(name="psL", bufs=2, space="PSUM") as pL,
            tc.tile_pool(name="psF", bufs=1, space="PSUM") as pF,
            tc.tile_pool(name="psV", bufs=4, space="PSUM") as pV,
        ):
            wT = cpool.tile([C, K], bf16, tag="wT")
            cstf = cpool.tile([C, 840], f32, tag="cstf")
            nc.sync.dma_start(wT[:], cb_in[:])
            nc.sync.dma_start(cstf[:], cf_in[:])
            b64 = cstf[:, 0:K]
            identF = cstf[:, 64:192]
            cen = cstf[0:KE, 192:320]
            gnl = cstf[:, 320:321]
            b512 = cstf[:, 328:328 + TPC * K]

            PT = NCH * TPC  # 32 pixel tiles per image

            psVs = []
            for img in range(NIMG):
                # image-level xT store: [p, (32 tiles, TS): 0:128]=xT, col 128=n
                xTs = tpool.tile([C, PT * TS], bf16, tag="xTs")
                xTs_v = xTs[:].rearrange("p (t r) -> p t r", r=TS)
                ssqI = spool.tile([C, PT], f32, tag="ssqI")
                for ch in range(NCH):
                    nc.sync.dma_start_transpose(
                        xTs_v[:, ch * TPC:(ch + 1) * TPC, 0:128],
                        x_in[img, :, ch * CH:(ch + 1) * CH])
                    xsq = wpool.tile([C, CH], bf16, tag="xsq")
                    xsq_v = xsq[:].rearrange("p (t r) -> p t r", r=128)
                    nc.scalar.activation(
                        xsq_v, xTs_v[:, ch * TPC:(ch + 1) * TPC, 0:128],
                        Act.Square)
                    nc.vector.tensor_reduce(
                        ssqI[:, ch * TPC:(ch + 1) * TPC], xsq_v,
                        axis=mybir.AxisListType.X, op=Alu.add)

                # per-image stats on ACT: 3 ops, one table-set pair
                lnssq = spool.tile([C, PT], f32, tag="lnssq")
                nc.scalar.activation(lnssq[:], ssqI[:], Act.Ln)
                invcI = spool.tile([C, PT], f32, tag="invcI")
                nc.scalar.activation(invcI[:], lnssq[:], Act.Exp, scale=-0.5)
                nc.scalar.activation(xTs_v[:, :, 128:129], lnssq[:],
                                     Act.Exp, scale=0.5)

                # vlad accumulator [0:56, 0:128]=term1, [:,128]=s_k
                psV = pV.tile([C, 512], f32, tag="psV", name=f"psV{img}")
                psVs.append(psV)
                for ch in range(NCH):
                    xin = xpool.tile([C, CH], bf16, tag="x")
                    nc.sync.dma_start(xin[:], x_in[img, :, ch * CH:(ch + 1) * CH])

                    invc = invcI[:, ch * TPC:(ch + 1) * TPC]

                    psumL = pL.tile([C, TPC * K], f32, tag="L")
                    for j in range(TPC):
                        nc.tensor.matmul(psumL[:, j * K:(j + 1) * K],
                                         xin[:, j * 128:(j + 1) * 128], wT[:],
                                         start=True, stop=True)

                    # lu = raw*invn (DVE big op from psum)
                    lu = wpool.tile([C, TPC * K], f32, tag="lu")
                    nc.vector.tensor_tensor(
                        lu[:].rearrange("p (t k) -> p t k", k=K),
                        psumL[:].rearrange("p (t k) -> p t k", k=K),
                        invc.broadcast_to([C, TPC, K]), Alu.mult)
                    # ll = lu + b (Pool)
                    ll = wpool.tile([C, TPC * K], f32, tag="ll")
                    nc.gpsimd.tensor_tensor(ll[:], lu[:], b512, Alu.add)
                    # -max over k per (pixel, tile)
                    nmcol = spool.tile([C, TPC], f32, tag="nmcol")
                    nc.vector.tensor_reduce(
                        nmcol[:], ll[:].rearrange("p (t k) -> p t k", k=K),
                        axis=mybir.AxisListType.X, op=Alu.max, negate=True)
                    # dd = ll - m (Pool)
                    dd = wpool.tile([C, TPC * K], f32, tag="dd")
                    nc.gpsimd.tensor_tensor(
                        dd[:].rearrange("p (t k) -> p t k", k=K),
                        ll[:].rearrange("p (t k) -> p t k", k=K),
                        nmcol[:].broadcast_to([C, TPC, K]), Alu.add)
                    # ee = exp(dd) one big ACT op
                    ee = wpool.tile([C, TPC * K], bf16, tag="ee")
                    nc.scalar.activation(ee[:], dd[:], Act.Exp)
                    # scol = sumexp (DVE)
                    scol = spool.tile([C, TPC], f32, tag="scol")
                    nc.vector.tensor_reduce(
                        scol[:], ee[:].rearrange("p (t k) -> p t k", k=K),
                        axis=mybir.AxisListType.X, op=Alu.add)
                    gcol = spool.tile([C, TPC], f32, tag="gcol")
                    nc.vector.reciprocal(gcol[:], scol[:])
                    rcol = spool.tile([C, TPC], f32, tag="rcol")
                    nc.gpsimd.tensor_tensor(rcol[:], invc, gcol[:], Alu.mult)
                    # a_r = ee * rcol (Pool, broadcast over k)
                    aa = wpool.tile([C, TPC * K], bf16, tag="aa")
                    nc.gpsimd.tensor_tensor(
                        aa[:].rearrange("p (t k) -> p t k", k=K),
                        ee[:].rearrange("p (t k) -> p t k", k=K),
                        rcol[:].broadcast_to([C, TPC, K]), Alu.mult)

                    if img == 0 and ch == 0:
                        nc.gpsimd.dma_start(dbg_ext[:, 0:TPC * K], aa[:])
                        nc.sync.dma_start(dbg_ext[:, 512:512 + TPC],
                                          ssqI[:, 0:TPC])
                        nc.sync.dma_start(dbg_ext[:, 528:528 + TPC], nmcol[:])
                        nc.sync.dma_start(dbg_ext[:, 536:536 + TPC], scol[:])

                    first = ch == 0
                    last = ch == NCH - 1
                    for j in range(TPC):
                        t = ch * TPC + j
                        nc.tensor.matmul(psV[0:KE, 0:129],
                                         aa[:, j * K:j * K + KE],
                                         xTs[:, t * TS:t * TS + 129],
                                         start=(first and j == 0),
                                         stop=(last and j == TPC - 1))


            # ---- batched per-image tails ----
            ssqk4 = spool.tile([C, NIMG], f32, tag="ssqk4")
            vks = []
            for img in range(NIMG):
                psV = psVs[img]
                negs = spool.tile([KE, 1], f32, tag="negs", name=f"negs{img}")
                nc.vector.tensor_scalar_mul(negs[:], psV[0:KE, 128:129], -1.0)
                vk = fpool.tile([KE, C], f32, tag="vk", name=f"vk{img}")
                nc.vector.scalar_tensor_tensor(vk[:], cen, negs[:],
                                               psV[0:KE, 0:C],
                                               Alu.mult, Alu.add)
                vks.append(vk)
            psA = pF.tile([C, 512], f32, tag="psA")
            for img in range(NIMG):
                nc.tensor.matmul(psA[:, img * 128:img * 128 + KE], vks[img][:],
                                 identF[0:KE, 0:KE],
                                 is_transpose=True, start=True, stop=True)
                trash_a = fpool.tile([C, KE], f32, tag="tra", name=f"tra{img}")
                nc.scalar.activation(trash_a[:],
                                     psA[:, img * 128:img * 128 + KE],
                                     Act.Square,
                                     accum_out=ssqk4[:, img:img + 1])
            lnk = spool.tile([C, NIMG], f32, tag="lnk")
            nc.scalar.activation(lnk[:], ssqk4[:], Act.Ln)
            comb = spool.tile([C, NIMG], f32, tag="comb")
            nc.scalar.activation(comb[:], lnk[:], Act.Exp,
                                 scale=-0.5, bias=gnl)
            psB = pF.tile([C, 512], f32, tag="psB")
            for img in range(NIMG):
                vnT = fpool.tile([C, KE], f32, tag="vnT", name=f"vnT{img}")
                nc.vector.tensor_scalar(vnT[:],
                                        psA[:, img * 128:img * 128 + KE],
                                        comb[:, img:img + 1], None, Alu.mult)
                nc.tensor.matmul(psB[0:KE, img * 128:img * 128 + C], vnT[:],
                                 identF,
                                 is_transpose=True, start=True, stop=True)
                ob = fpool.tile([KE, C], f32, tag="ob", name=f"ob{img}")
                nc.scalar.activation(ob[:],
                                     psB[0:KE, img * 128:img * 128 + C],
                                     Act.Copy)
                nc.sync.dma_start(out_ext[img], ob[:])

    nc.compile()
    return nc


def _get_nc():
    if "nc" not in _cache:
        _cache["nc"] = _build()
    return _cache["nc"]


def _make_in_maps(x, conv_w, conv_b, centroids, n_cores=8):
    x = np.asarray(x, dtype=np.float32)
    conv_w = np.asarray(conv_w, dtype=np.float32)
    conv_b = np.asarray(conv_b, dtype=np.float32)
    centroids = np.asarray(centroids, dtype=np.float32)

    N = x.shape[0]
    per = N // n_cores
    assert per == NIMG

    xr = np.ascontiguousarray(
        x.reshape(N, C, P).astype(ml_dtypes.bfloat16))

    cstb = np.ascontiguousarray(conv_w.T.astype(ml_dtypes.bfloat16))

    cstf = np.zeros((C, 840), dtype=np.float32)
    cstf[:, 0:K] = conv_b[None, :]
    cstf[:, 64:192] = np.eye(C, dtype=np.float32)
    cstf[0:KE, 192:320] = centroids[:KE]
    cstf[:, 320] = -0.5 * np.log(128.0)
    cstf[:, 328:328 + 512] = np.tile(conv_b, 8)[None, :]

    in_maps = []
    for i in range(n_cores):
        in_maps.append({
            "x": np.ascontiguousarray(xr[i * per:(i + 1) * per]),
            "cstb": cstb,
            "cstf": cstf,
        })
    return in_maps


def kernel(x, conv_w, conv_b, centroids):
    from concourse.bass_utils import run_bass_kernel_spmd

    n_cores = 8
    per = np.asarray(x).shape[0] // n_cores
    in_maps = _make_in_maps(x, conv_w, conv_b, centroids, n_cores)

    nc = _get_nc()
    res = run_bass_kernel_spmd(nc, in_maps, list(range(n_cores)))
    outs = [np.asarray(r["out"]).reshape(per, KE * C) for r in res.results]
    return np.concatenate(outs, axis=0)


if __name__ == "__main__":
    rng = np.random.default_rng(0)
    x = rng.standard_normal((32, C, 64, 64), dtype=np.float32)
    w = rng.standard_normal((K, C), dtype=np.float32)
    b = rng.standard_normal((K,), dtype=np.float32)
    c = rng.random((K, C), dtype=np.float32)
    out = kernel(x=x, conv_w=w, conv_b=b, centroids=c)
    print(out.shape, out.dtype)


# revision 20
# speedup vs baseline: 1.9212x; 1.0501x over previous
"""NetVLAD Trainium2 kernel — data-parallel over N across 8 cores, bf16.

Per core: 4 images [C=128, P=4096].  x is loaded twice per 1024-px chunk:
once plain [c, p] (logits stationaries) and once via HWDGE DMA-transpose
[p, c] straight into SBUF (no PE transpose, no psum evict).

Per chunk (8 tiles of 128 px):
  SP:   plain DMA + transpose DMA.
  Pool: xsq = xT*xT; rcol = invn/sumexp; a_r = ee * rcol.
  DVE:  ssq 3D-reduce; fused nll=-(u+b) with accum min -> -max(l); recip.
  ACT:  invn/n via exp(+-0.5*ln(ssq)) (ln+exp share one act table -> no
        table thrash; n lands directly in the xTs 129th column);
        u = raw*invn scaled-copy from psum; ee = exp(-nll - m) + sumexp.
  PE:   8x logits matmul (stationary x_tile bf16, moving wT 64 cols),
        8x vlad matmul (stationary a_r[:, :56], moving [xT | n] 129 cols)
        accumulated into psum [56, 129] over the image.
Tail per image: vk = term1 - s*cen, PE transpose, intra-norm via
exp(-0.5 ln(ssq_k)) * 1/sqrt(128) (the global norm is exactly 1/sqrt(128)
because every intra-normalized column has unit norm), transpose back, DMA.
"""

import sys

for _p in ("/opt/trn_rl_repo",):
    if _p not in sys.path:
        sys.path.insert(0, _p)

import numpy as np
import ml_dtypes

NIMG = 4      # images per core
C = 128
K = 64
KE = 56
P = 4096
TPC = 8       # 128-px tiles per chunk
CH = TPC * 128
NCH = P // CH  # 4 chunks per image
TS = 144      # xTs per-tile stride (bf16 elems); 288B = 32B-aligned

_cache = {}


def _build():
    import concourse.mybir as mybir
    from concourse import bacc, tile

    f32 = mybir.dt.float32
    bf16 = mybir.dt.bfloat16
    Alu = mybir.AluOpType
    Act = mybir.ActivationFunctionType

    nc = bacc.Bacc()
    x_in = nc.declare_dram_parameter("x", [NIMG, C, P], bf16, isOutput=False)
    cb_in = nc.declare_dram_parameter("cstb", [C, K], bf16, isOutput=False)
    # f32 consts: b64 [0:64] | identF [64:192] | cen rows0:56 [192:320]
    # | ln(1/sqrt(128)) [320]
    cf_in = nc.declare_dram_parameter("cstf", [C, 840], f32, isOutput=False)
    out_ext = nc.declare_dram_parameter("out", [NIMG, KE, C], f32, isOutput=True)
    dbg_ext = nc.declare_dram_parameter("dbg", [C, 704], f32, isOutput=True)

    with tile.TileContext(nc) as tc:
        with (
            tc.tile_pool(name="const", bufs=1) as cpool,
            tc.tile_pool(name="xin", bufs=4) as xpool,
            tc.tile_pool(name="xts", bufs=2) as tpool,
            tc.tile_pool(name="work", bufs=3) as wpool,
            tc.tile_pool(name="stats", bufs=4) as spool,
            tc.tile_pool(name="fin", bufs=2) as fpool,
            tc.tile_pool(name="psL", bufs=2, space="PSUM") as pL,
            tc.tile_pool(name="psF", bufs=1, space="PSUM") as pF,
            tc.tile_pool(name="psV", bufs=4, space="PSUM") as pV,
        ):
            wT = cpool.tile([C, K], bf16, tag="wT")
            cstf = cpool.tile([C, 840], f32, tag="cstf")
            nc.sync.dma_start(wT[:], cb_in[:])
            nc.sync.dma_start(cstf[:], cf_in[:])
            b64 = cstf[:, 0:K]
            identF = cstf[:, 64:192]
            cen = cstf[0:KE, 192:320]
            gnl = cstf[:, 320:321]
            b512 = cstf[:, 328:328 + TPC * K]

            PT = NCH * TPC  # 32 pixel tiles per image

            psVs = []
            for img in range(NIMG):
                # image-level xT store: [p, (32 tiles, TS): 0:128]=xT, col 128=n
                xTs = tpool.tile([C, PT * TS], bf16, tag="xTs")
                xTs_v = xTs[:].rearrange("p (t r) -> p t r", r=TS)
                ssqI = spool.tile([C, PT], f32, tag="ssqI")
                for ch in range(NCH):
                    nc.sync.dma_start_transpose(
                        xTs_v[:, ch * TPC:(ch + 1) * TPC, 0:128],
                        x_in[img, :, ch * CH:(ch + 1) * CH])
                    xsq = wpool.tile([C, CH], bf16, tag="xsq")
                    xsq_v = xsq[:].rearrange("p (t r) -> p t r", r=128)
                    nc.scalar.activation(
                        xsq_v, xTs_v[:, ch * TPC:(ch + 1) * TPC, 0:128],
                        Act.Square)
                    nc.vector.tensor_reduce(
                        ssqI[:, ch * TPC:(ch + 1) * TPC], xsq_v,
                        axis=mybir.AxisListType.X, op=Alu.add)

                # per-image stats on ACT: 3 ops, one table-set pair
                lnssq = spool.tile([C, PT], f32, tag="lnssq")
                nc.scalar.activation(lnssq[:], ssqI[:], Act.Ln)
                invcI = spool.tile([C, PT], f32, tag="invcI")
                nc.scalar.activation(invcI[:], lnssq[:], Act.Exp, scale=-0.5)
                nc.scalar.activation(xTs_v[:, :, 128:129], lnssq[:],
                                     Act.Exp, scale=0.5)

                # vlad accumulator [0:56, 0:128]=term1, [:,128]=s_k
                psV = pV.tile([C, 512], f32, tag="psV", name=f"psV{img}")
                psVs.append(psV)
                for ch in range(NCH):
                    xin = xpool.tile([C, CH], bf16, tag="x")
                    nc.sync.dma_start(xin[:], x_in[img, :, ch * CH:(ch + 1) * CH])

                    invc = invcI[:, ch * TPC:(ch + 1) * TPC]

                    psumL = pL.tile([C, TPC * K], f32, tag="L")
                    for j in range(TPC):
                        nc.tensor.matmul(psumL[:, j * K:(j + 1) * K],
                                         xin[:, j * 128:(j + 1) * 128], wT[:],
                                         start=True, stop=True)

                    # lu = raw*invn (DVE big op from psum)
                    lu = wpool.tile([C, TPC * K], f32, tag="lu")
                    nc.vector.tensor_tensor(
                        lu[:].rearrange("p (t k) -> p t k", k=K),
                        psumL[:].rearrange("p (t k) -> p t k", k=K),
                        invc.broadcast_to([C, TPC, K]), Alu.mult)
                    # ll = lu + b (Pool)
                    ll = wpool.tile([C, TPC * K], f32, tag="ll")
                    nc.gpsimd.tensor_tensor(ll[:], lu[:], b512, Alu.add)
                    # -max over k per (pixel, tile)
                    nmcol = spool.tile([C, TPC], f32, tag="nmcol")
                    nc.vector.tensor_reduce(
                        nmcol[:], ll[:].rearrange("p (t k) -> p t k", k=K),
                        axis=mybir.AxisListType.X, op=Alu.max, negate=True)
                    # dd = ll - m (Pool)
                    dd = wpool.tile([C, TPC * K], f32, tag="dd")
                    nc.gpsimd.tensor_tensor(
                        dd[:].rearrange("p (t k) -> p t k", k=K),
                        ll[:].rearrange("p (t k) -> p t k", k=K),
                        nmcol[:].broadcast_to([C, TPC, K]), Alu.add)
                    # ee = exp(dd) one big ACT op
                    ee = wpool.tile([C, TPC * K], bf16, tag="ee")
                    nc.scalar.activation(ee[:], dd[:], Act.Exp)
                    # scol = sumexp (DVE)
                    scol = spool.tile([C, TPC], f32, tag="scol")
                    nc.vector.tensor_reduce(
                        scol[:], ee[:].rearrange("p (t k) -> p t k", k=K),
                        axis=mybir.AxisListType.X, op=Alu.add)
                    gcol = spool.tile([C, TPC], f32, tag="gcol")
                    nc.vector.reciprocal(gcol[:], scol[:])
                    rcol = spool.tile([C, TPC], f32, tag="rcol")
                    nc.gpsimd.tensor_tensor(rcol[:], invc, gcol[:], Alu.mult)
                    # a_r = ee * rcol (Pool, broadcast over k)
                    aa = wpool.tile([C, TPC * K], bf16, tag="aa")
                    nc.gpsimd.tensor_tensor(
                        aa[:].rearrange("p (t k) -> p t k", k=K),
                        ee[:].rearrange("p (t k) -> p t k", k=K),
                        rcol[:].broadcast_to([C, TPC, K]), Alu.mult)

                    if img == 0 and ch == 0:
                        nc.gpsimd.dma_start(dbg_ext[:, 0:TPC * K], aa[:])
                        nc.sync.dma_start(dbg_ext[:, 512:512 + TPC],
                                          ssqI[:, 0:TPC])
                        nc.sync.dma_start(dbg_ext[:, 528:528 + TPC], nmcol[:])
                        nc.sync.dma_start(dbg_ext[:, 536:536 + TPC], scol[:])

                    first = ch == 0
                    last = ch == NCH - 1
                    for j in range(TPC):
                        t = ch * TPC + j
                        nc.tensor.matmul(psV[0:KE, 0:129],
                                         aa[:, j * K:j * K + KE],
                                         xTs[:, t * TS:t * TS + 129],
                                         start=(first and j == 0),
                                         stop=(last and j == TPC - 1))


            # ---- batched per-image tails ----
            ssqk4 = spool.tile([C, NIMG], f32, tag="ssqk4")
            vks = []
            for img in range(NIMG):
                psV = psVs[img]
                negs = spool.tile([KE, 1], f32, tag="negs", name=f"negs{img}")
                nc.vector.tensor_scalar_mul(negs[:], psV[0:KE, 128:129], -1.0)
                vk = fpool.tile([KE, C], f32, tag="vk", name=f"vk{img}")
                nc.vector.scalar_tensor_tensor(vk[:], cen, negs[:],
                                               psV[0:KE, 0:C],
                                               Alu.mult, Alu.add)
                vks.append(vk)
            psA = pF.tile([C, 512], f32, tag="psA")
            for img in range(NIMG):
                nc.tensor.matmul(psA[:, img * 128:img * 128 + KE], vks[img][:],
                                 identF[0:KE, 0:KE],
                                 is_transpose=True, start=True, stop=True)
                trash_a = fpool.tile([C, KE], f32, tag="tra", name=f"tra{img}")
                nc.scalar.activation(trash_a[:],
                                     psA[:, img * 128:img * 128 + KE],
                                     Act.Square,
                                     accum_out=ssqk4[:, img:img + 1])
            lnk = spool.tile([C, NIMG], f32, tag="lnk")
            nc.scalar.activation(lnk[:], ssqk4[:], Act.Ln)
            comb = spool.tile([C, NIMG], f32, tag="comb")
            nc.scalar.activation(comb[:], lnk[:], Act.Exp,
                                 scale=-0.5, bias=gnl)
            psB = pF.tile([C, 512], f32, tag="psB")
            for img in range(NIMG):
                vnT = fpool.tile([C, KE], f32, tag="vnT", name=f"vnT{img}")
                nc.vector.tensor_scalar(vnT[:],
                                        psA[:, img * 128:img * 128 + KE],
                                        comb[:, img:img + 1], None, Alu.mult)
                nc.tensor.matmul(psB[0:KE, img * 128:img * 128 + C], vnT[:],
                                 identF,
                                 is_transpose=True, start=True, stop=True)
                ob = fpool.tile([KE, C], f32, tag="ob", name=f"ob{img}")
                nc.scalar.activation(ob[:],
                                     psB[0:KE, img * 128:img * 128 + C],
                                     Act.Copy)
                nc.sync.dma_start(out_ext[img], ob[:])

    nc.compile()
    return nc


def _get_nc():
    if "nc" not in _cache:
        _cache["nc"] = _build()
    return _cache["nc"]


def _make_in_maps(x, conv_w, conv_b, centroids, n_cores=8):
    x = np.asarray(x, dtype=np.float32)
    conv_w = np.asarray(conv_w, dtype=np.float32)
    conv_b = np.asarray(conv_b, dtype=np.float32)
    centroids = np.asarray(centroids, dtype=np.float32)

    N = x.shape[0]
    per = N // n_cores
    assert per == NIMG

    xr = np.ascontiguousarray(
        x.reshape(N, C, P).astype(ml_dtypes.bfloat16))

    cstb = np.ascontiguousarray(conv_w.T.astype(ml_dtypes.bfloat16))

    cstf = np.zeros((C, 840), dtype=np.float32)
    cstf[:, 0:K] = conv_b[None, :]
    cstf[:, 64:192] = np.eye(C, dtype=np.float32)
    cstf[0:KE, 192:320] = centroids[:KE]
    cstf[:, 320] = -0.5 * np.log(128.0)
    cstf[:, 328:328 + 512] = np.tile(conv_b, 8)[None, :]

    in_maps = []
    for i in range(n_cores):
        in_maps.append({
            "x": np.ascontiguousarray(xr[i * per:(i + 1) * per]),
            "cstb": cstb,
            "cstf": cstf,
        })
    return in_maps


def kernel(x, conv_w, conv_b, centroids):
    from concourse.bass_utils import run_bass_kernel_spmd

    n_cores = 8
    per = np.asarray(x).shape[0] // n_cores
    in_maps = _make_in_maps(x, conv_w, conv_b, centroids, n_cores)

    nc = _get_nc()
    res = run_bass_kernel_spmd(nc, in_maps, list(range(n_cores)))
    outs = [np.asarray(r["out"]).reshape(per, KE * C) for r in res.results]
    return np.concatenate(outs, axis=0)


if __name__ == "__main__":
    rng = np.random.default_rng(0)
    x = rng.standard_normal((32, C, 64, 64), dtype=np.float32)
    w = rng.standard_normal((K, C), dtype=np.float32)
    b = rng.standard_normal((K,), dtype=np.float32)
    c = rng.random((K, C), dtype=np.float32)
    out = kernel(x=x, conv_w=w, conv_b=b, centroids=c)
    print(out.shape, out.dtype)
